# revision 35
# baseline (speedup 1.0000x reference)
"""Trainium2 Bass kernel for nn_ContinusConvolution (GNN message passing).

Math (see reference):
    P   = s_ij @ W_s                     # (B,N,NB,C)
    G   = z_ij @ W_z                     # (B,N,NB,C)
    s1  = sum_k m_k * (P_k * G_k)        # (B,N,C)
    SG  = (sum_k m_k * z_k) @ W_z        # (B,N,C)   [= sum_k m_k G_k]
    out = LayerNorm(s1 - s_i * SG) * gamma + beta

Device mapping (per core, nodes sharded 8 ways over B*N):
    - Activations cast to bf16 host-side; fp32 accumulation in PSUM.
    - s_ij / z_ij loaded TRANSPOSED (contraction dim on partitions) via the
      DMA xbar transpose straight from DRAM -> no on-chip transposes.
    - Token tile = 128 nodes x 1 neighbor; projections use the transposed
      activation tile as the matmul stationary, weights as moving.
    - Mask m_k folds into the ScalarE PSUM->SBUF copy of G as a
      per-partition activation scale (free).
    - Masked k-reduction = identity-weight matmuls accumulating each
      P_k*G'_k tile into one PSUM bank (start/stop over the 32 k's).
    - sum_k m_k z_k is tiny and data-linear: computed host-side, projected
      through W_z on device for the s_i * SG term.
    - LayerNorm via bn_stats/bn_aggr + Sqrt/reciprocal.
"""

import contextlib

import numpy as np
import ml_dtypes

import concourse.bass as bass
import concourse.mybir as mybir
import concourse.tile as tile
from concourse import bacc
from concourse.bass_utils import run_bass_kernel_spmd

B, N, NB, C, CZ = 4, 1024, 32, 384, 128
EPS = 1e-6
NCORES = 8
NODES = B * N                      # 4096 total nodes
NPC = NODES // NCORES              # 512 nodes per core
PGROUP = 128                       # nodes per group (partition dim)
BLOCKS = 4                         # node blocks of 32 per group
KW = 4                             # k-window per tile
KCH = NB // KW                     # 8 k-chunks
CE = C // 128                      # 3 c-chunks

bf16 = ml_dtypes.bfloat16
dt = mybir.dt


def build_nc(groups=NPC // PGROUP, probe=False, reps=1):
    nodes = groups * PGROUP
    nc = bacc.Bacc("TRN2", target_bir_lowering=False, debug=False)

    s_t = nc.declare_dram_parameter("s_t", [nodes, NB * C], dt.bfloat16, isOutput=False)
    z_t = nc.declare_dram_parameter("z_t", [nodes * NB, CZ], dt.bfloat16, isOutput=False)
    szt = nc.declare_dram_parameter("szt", [nodes, CZ], dt.bfloat16, isOutput=False)
    mcl = nc.declare_dram_parameter("mcl", [nodes, NB], dt.float32, isOutput=False)
    idn = nc.declare_dram_parameter("idn", [128, 128], dt.bfloat16, isOutput=False)
    s_i = nc.declare_dram_parameter("s_i", [nodes, C], dt.float32, isOutput=False)
    w_s = nc.declare_dram_parameter("w_s", [CE, 128, C], dt.bfloat16, isOutput=False)
    w_z = nc.declare_dram_parameter("w_z", [CZ, C], dt.bfloat16, isOutput=False)
    gmb = nc.declare_dram_parameter("gmb", [2, 128, C], dt.float32, isOutput=False)
    out = nc.declare_dram_parameter("out", [nodes, C], dt.float32, isOutput=True)
    if probe:
        pr_s1 = nc.declare_dram_parameter("pr_s1", [nodes, C], dt.float32, isOutput=True)
        pr_sg = nc.declare_dram_parameter("pr_sg", [nodes, C], dt.float32, isOutput=True)
        pr_p = nc.declare_dram_parameter("pr_p", [128, C], dt.float32, isOutput=True)
        pr_g = nc.declare_dram_parameter("pr_g", [128, C], dt.float32, isOutput=True)
        pr_t = nc.declare_dram_parameter("pr_t", [128, C], dt.float32, isOutput=True)

    with tile.TileContext(nc) as tc:
        with (
            tc.tile_pool(name="const", bufs=1) as cpool,
            tc.tile_pool(name="sT", bufs=2) as sT_pool,
            tc.tile_pool(name="zT", bufs=2) as zT_pool,
            tc.tile_pool(name="mcp", bufs=2) as mc_pool,
            tc.tile_pool(name="sip", bufs=2) as si_pool,
            tc.tile_pool(name="gsb", bufs=3) as gsb_pool,
            tc.tile_pool(name="tsb", bufs=3) as tsb_pool,
            tc.tile_pool(name="epi", bufs=2) as epi_pool,
            tc.tile_pool(name="outp", bufs=2) as out_pool,
            tc.tile_pool(name="psum_p", bufs=2, space="PSUM") as p_pool,
            tc.tile_pool(name="psum_g", bufs=3, space="PSUM") as g_pool,
            tc.tile_pool(name="psum_acc", bufs=1, space="PSUM") as acc_pool,
        ):
            wssb = cpool.tile([128, CE, C], dt.bfloat16)
            nc.sync.dma_start(out=wssb, in_=w_s[:].rearrange("e p d -> p e d"))
            wzsb = cpool.tile([128, C], dt.bfloat16)
            nc.sync.dma_start(out=wzsb, in_=w_z[:])
            gam = cpool.tile([128, C], dt.float32)
            nc.sync.dma_start(out=gam, in_=gmb[0])
            bet = cpool.tile([128, C], dt.float32)
            nc.sync.dma_start(out=bet, in_=gmb[1])
            iden = cpool.tile([128, 128], dt.bfloat16)
            nc.sync.dma_start(out=iden, in_=idn[:])
            epst = cpool.tile([128, 1], dt.float32)
            nc.vector.memset(epst, EPS)
            # SZ^T: [cz, node] = transpose of host-reduced sum_k m_k z_k
            sztsb = cpool.tile([128, nodes], dt.bfloat16)
            nc.sync.dma_start_transpose(sztsb, szt[:])

            KB = 2  # k's per super-tile (product/copy batching)
            loop_cm = tc.For_i(0, reps, 1) if reps > 1 else contextlib.nullcontext()
            with loop_cm:
              for g in range(groups):
                  gsl = slice(g * 128, (g + 1) * 128)
                  # sT[p, k, e, n] = s[node g*128+n, k, e*128+p]
                  sTg = sT_pool.tile([128, NB, CE, 128], dt.bfloat16)
                  nc.sync.dma_start_transpose(sTg, s_t[gsl, :])
                  # zT[p, n, k] = z[node g*128+n, k, p]; xbar writes the flat
                  # [128, rows] form (row = n*NB + k), viewed here as [p, n, k]
                  zTg = zT_pool.tile([128, 128, NB], dt.bfloat16)
                  nc.sync.dma_start_transpose(
                      zTg.rearrange("p n k -> p (n k)"),
                      z_t[g * 128 * NB:(g + 1) * 128 * NB, :],
                  )
                  mcg = mc_pool.tile([128, NB], dt.float32)
                  nc.sync.dma_start(out=mcg, in_=mcl[gsl, :])
                  sig = si_pool.tile([128, C], dt.float32)
                  nc.sync.dma_start(out=sig, in_=s_i[gsl, :])

                  S1 = acc_pool.tile([128, C], dt.float32)
                  for kt in range(NB // KB):
                      # token tile = all 128 nodes of the group, one k each
                      P2 = p_pool.tile([128, KB, 512], dt.float32)
                      gsb2 = gsb_pool.tile([128, KB, C], dt.bfloat16)
                      for kk in range(KB):
                          k = kt * KB + kk
                          for e in range(CE):
                              nc.tensor.matmul(
                                  P2[:, kk, :C], sTg[:, k, e, :], wssb[:, e, :],
                                  start=(e == 0), stop=(e == CE - 1),
                              )
                          G = g_pool.tile([128, C], dt.float32)
                          nc.tensor.matmul(G, zTg[:, :, k], wzsb, start=True, stop=True)
                          # masked PSUM->SBUF copy: gsb = m[:, k] * G
                          nc.scalar.activation(
                              out=gsb2[:, kk, :], in_=G,
                              func=mybir.ActivationFunctionType.Copy,
                              scale=mcg[:, k:k + 1],
                          )
                      t2 = tsb_pool.tile([128, KB, C], dt.bfloat16)
                      nc.vector.tensor_tensor(
                          out=t2, in0=P2[:, :, :C], in1=gsb2, op=mybir.AluOpType.mult
                      )
                      if probe and g == 0 and kt == 0:
                          pb = epi_pool.tile([128, C], dt.float32, tag="pb")
                          nc.vector.tensor_copy(out=pb, in_=P2[:, 0, :C])
                          nc.sync.dma_start(out=pr_p[:], in_=pb)
                          pb2 = epi_pool.tile([128, C], dt.float32, tag="pb2")
                          nc.vector.tensor_copy(out=pb2, in_=gsb2[:, 0, :])
                          nc.sync.dma_start(out=pr_g[:], in_=pb2)
                          pb3 = epi_pool.tile([128, C], dt.float32, tag="pb3")
                          nc.vector.tensor_copy(out=pb3, in_=t2[:, 0, :])
                          nc.sync.dma_start(out=pr_t[:], in_=pb3)
                      # S1 += T'_k via identity matmul PSUM accumulation
                      for kk in range(KB):
                          nc.tensor.matmul(
                              S1, iden, t2[:, kk, :],
                              start=(kt == 0 and kk == 0),
                              stop=(kt == NB // KB - 1 and kk == KB - 1),
                              skip_group_check=True,
                          )

                  # ---- group epilogue ----
                  SG = g_pool.tile([128, C], dt.float32, tag="G")
                  nc.tensor.matmul(
                      SG, sztsb[:, g * 128:(g + 1) * 128], wzsb,
                      start=True, stop=True,
                  )
                  if probe:
                      pb4 = epi_pool.tile([128, C], dt.float32, tag="pb4")
                      nc.vector.tensor_copy(out=pb4, in_=S1)
                      nc.sync.dma_start(out=pr_s1[g * 128:(g + 1) * 128, :], in_=pb4)
                      pb5 = epi_pool.tile([128, C], dt.float32, tag="pb5")
                      nc.vector.tensor_copy(out=pb5, in_=SG)
                      nc.sync.dma_start(out=pr_sg[g * 128:(g + 1) * 128, :], in_=pb5)
                  tmp = epi_pool.tile([128, C], dt.float32)
                  nc.vector.tensor_tensor(
                      out=tmp, in0=SG, in1=sig, op=mybir.AluOpType.mult
                  )
                  pre = out_pool.tile([128, C], dt.float32)
                  nc.vector.tensor_tensor(
                      out=pre, in0=S1, in1=tmp, op=mybir.AluOpType.subtract
                  )
                  stats = epi_pool.tile([128, 6], dt.float32)
                  nc.vector.bn_stats(out=stats, in_=pre)
                  mv = epi_pool.tile([128, 2], dt.float32)
                  nc.vector.bn_aggr(out=mv, in_=stats)
                  rstd = epi_pool.tile([128, 1], dt.float32)
                  nc.scalar.activation(
                      out=rstd, in_=mv[:, 1:2],
                      func=mybir.ActivationFunctionType.Sqrt,
                      bias=epst, scale=1.0,
                  )
                  nc.vector.reciprocal(out=rstd, in_=rstd)
                  nc.vector.tensor_scalar(
                      out=pre, in0=pre,
                      scalar1=mv[:, 0:1], scalar2=rstd,
                      op0=mybir.AluOpType.subtract, op1=mybir.AluOpType.mult,
                  )
                  nc.vector.tensor_mul(pre, pre, gam)
                  nc.vector.tensor_add(pre, pre, bet)
                  nc.sync.dma_start(out=out[g * 128:(g + 1) * 128, :], in_=pre)

    nc.compile()
    return nc


def host_prep(s_i, s_ij, m_ij, z_ij, W_s, W_z, gamma, beta, groups=NPC // PGROUP):
    """Build per-core input maps (all numpy, cheap linear prep)."""
    nodes_pc = groups * PGROUP
    s_flat = np.ascontiguousarray(s_ij.reshape(NODES, NB * C))
    z_flat = np.ascontiguousarray(z_ij.reshape(NODES * NB, CZ))
    si_flat = np.ascontiguousarray(s_i.reshape(NODES, C)).astype(np.float32)
    m_flat = m_ij.reshape(NODES, NB).astype(np.float32)
    # host-side masked k-sum of z (tiny, linear in input size)
    sz = np.einsum("nk,nkz->nz", m_flat, z_ij.reshape(NODES, NB, CZ)).astype(bf16)

    w_s_h = np.ascontiguousarray(W_s.reshape(CE, 128, C)).astype(bf16)
    w_z_h = np.ascontiguousarray(W_z).astype(bf16)
    gmb_h = np.stack([
        np.broadcast_to(gamma.astype(np.float32), (128, C)),
        np.broadcast_to(beta.astype(np.float32), (128, C)),
    ]).copy()

    iden = np.eye(128, dtype=np.float32).astype(bf16)
    in_maps = []
    for c in range(NCORES):
        lo = c * NPC
        nsl = slice(lo, lo + nodes_pc)
        in_maps.append({
            "s_t": s_flat[nsl].astype(bf16),
            "z_t": z_flat[lo * NB:(lo + nodes_pc) * NB].astype(bf16),
            "szt": sz[nsl],
            "mcl": np.ascontiguousarray(m_flat[nsl]),
            "idn": iden,
            "s_i": si_flat[nsl],
            "w_s": w_s_h,
            "w_z": w_z_h,
            "gmb": gmb_h,
        })
    return in_maps


_NC_CACHE = {}


def _get_nc(groups):
    if groups not in _NC_CACHE:
        _NC_CACHE[groups] = build_nc(groups)
    return _NC_CACHE[groups]


def kernel(s_i, s_ij, m_ij, z_ij, W_s, W_z, gamma, beta):
    s_i = np.asarray(s_i)
    s_ij = np.asarray(s_ij)
    m_ij = np.asarray(m_ij)
    z_ij = np.asarray(z_ij)
    W_s = np.asarray(W_s)
    W_z = np.asarray(W_z)
    gamma = np.asarray(gamma)
    beta = np.asarray(beta)

    nc = _get_nc(NPC // PGROUP)
    in_maps = host_prep(s_i, s_ij, m_ij, z_ij, W_s, W_z, gamma, beta)
    res = run_bass_kernel_spmd(
        nc, in_maps, list(range(NCORES)), trace=TRACE, **TRACE_KWARGS
    )
    global LAST_RESULTS
    LAST_RESULTS = res
    outs = [np.asarray(res.results[i]["out"]) for i in range(NCORES)]
    return np.concatenate(outs, axis=0).reshape(B, N, C).astype(np.float32)


TRACE = False
TRACE_KWARGS = {}
LAST_RESULTS = None

